# revision 31
# baseline (speedup 1.0000x reference)
"""Causal multi-head attention (B=1, S=4096, D=768, H=12, d_head=64) on 8
Trainium2 NeuronCores.

Sharding: exact 1.5 heads per core. Slot A = head c (c = core id 0..7), full
causal attention over all 4096 queries. Slot B = head 8 + c//2 restricted to
query tokens of parity c%2 (2048 alternate tokens, full key range), so the 4
remaining heads are each split across two cores by query parity with zero
duplicated work and a uniform SPMD program (the parity lives in the data:
host-gathered xB rows and a parity-dependent boundary mask).

All matmul operands are bf16 (PSUM accumulation stays f32); the host supplies
x already transposed (and parity-gathered for slot B), so the device does no
x transposes at all. Per query tile the kernel interleaves next-tile QKV
projections and previous-tile out-projections into the attention block loop to
keep the PE busy (and at full clock) while the Scalar engine runs the exps.
Softmax denominators come free via ones-columns appended to V; normalization
uses reciprocal_approx_fast on DVE; out-proj PSUM->SBUF copies run on GpSimd.
Partial outputs are written bf16; the host sums them (the all-reduce of the
row-parallel out projection) and adds b_out.
"""

import sys

sys.path.insert(0, "/opt/trn_rl_repo")

from collections import deque

import ml_dtypes
import numpy as np

import concourse.bass as bass
import concourse.tile as tile
from concourse import bacc, mybir
from concourse.bass_utils import run_bass_kernel_spmd

S = 4096
D = 768
HD = 64
P = 128
KC = D // P  # 6 contraction chunks for the projections
NT = 8  # 512-token query tiles
NEG = -1e30

F32 = mybir.dt.float32
BF16 = mybir.dt.bfloat16
AF = mybir.ActivationFunctionType
ADD = mybir.AluOpType.add
MULT = mybir.AluOpType.mult

_CACHED_NC = None


def build_nc():
    nc = bacc.Bacc("TRN2", target_bir_lowering=False, debug=False, num_devices=8)

    xt_d = nc.declare_dram_parameter("xt", [8 * P, KC, 512], BF16, isOutput=False)
    xb_d = nc.declare_dram_parameter("xb", [4 * P, KC, 512], BF16, isOutput=False)
    # all constants packed in one bf16 tensor: [ident 128 | ma 128 | mb 64 |
    # wq 768 | wk 768 | wv 768 | wo 768 | woz 768] = 4160 columns -> one DMA
    # wave of 128 descriptors (descriptor dispatch is ~14.5ns each); woz is
    # [woA ; zeros] for full-rate K=128 solo out-projections
    cst_d = nc.declare_dram_parameter("cst", [P, 4160], BF16, isOutput=False)
    outa_d = nc.declare_dram_parameter("outA", [S, D], BF16, isOutput=True)
    outb_d = nc.declare_dram_parameter("outB", [S // 2, D], BF16, isOutput=True)

    with tile.TileContext(nc) as tc:
        with (
            tc.tile_pool(name="const", bufs=1) as const,
            tc.tile_pool(name="big", bufs=1) as big,
            tc.tile_pool(name="pt", bufs=8) as ptp,
            tc.tile_pool(name="vt", bufs=2) as vtp,
            tc.tile_pool(name="osb", bufs=6) as osbp,
            tc.tile_pool(name="sm", bufs=2) as sm,
            tc.tile_pool(name="ps", bufs=4, space="PSUM") as ps,
            tc.tile_pool(name="ctxA", bufs=2, space="PSUM") as ctxAp,
            tc.tile_pool(name="ctxB", bufs=1, space="PSUM") as ctxBp,
            tc.tile_pool(name="tpp", bufs=1, space="PSUM") as tpp,
        ):
            # ---- constants ----
            # Split each load 4 ways by partition: one queue processes
            # descriptors serially (~150ns each), so a 128-partition DMA on a
            # single queue takes ~19us; 4 queues cut that to ~5us.
            def dma4(dst, src):
                for q4 in range(4):
                    nc.sync.dma_start(dst[32 * q4 : 32 * (q4 + 1)],
                                      src[32 * q4 : 32 * (q4 + 1)])

            cst_s = const.tile([P, 4160], BF16)
            identb = cst_s[:, 0:128]
            ma_s = cst_s[:, 128:256]
            mb_s = cst_s[:, 256:320]
            wq_s = cst_s[:, 320:1088]
            wk_s = cst_s[:, 1088:1856]
            wv_s = cst_s[:, 1856:2624]
            wo_s = cst_s[:, 2624:3392]
            woz_s = cst_s[:, 3392:4160]

            # ---- persistent activations ----
            xTs = big.tile([P, NT, KC, 512], BF16)  # x^T, group-major
            xBs = big.tile([P, 4, KC, 512], BF16)  # x^T of slot-B tokens
            qT = big.tile([P, S], BF16)  # rows 0:64 qA^T, 64:128 qB^T (cols 0:2048)
            k2 = big.tile([P, S], BF16)  # rows 0:64 kA^T, 64:128 kB^T
            # v natural per 128-key block: cols 0:64 vA, 64 ones, 66:130 vB,
            # 130 ones (65/131 unused)
            vNat = big.tile([P, S // P, 132], BF16)
            cT = big.tile([P, S], BF16)  # rows 0:64 ctxA^T, 64:128 ctxB^T
            # kA with zeroed upper rows: solo A-scores contract K=128 at full
            # stream rate (K=64 matmuls stream at half rate unless paired)
            k2z = big.tile([P, S], BF16)

            def dma_x(dst, src, g):
                for q4 in range(4):
                    nc.sync.dma_start(
                        dst[32 * q4 : 32 * (q4 + 1), g, :, :],
                        src[P * g + 32 * q4 : P * g + 32 * (q4 + 1), :, :],
                    )

            # Critical wave: everything tile group 0 needs, first in the
            # queues.
            dma4(cst_s, cst_d)
            dma_x(xTs, xt_d, 0)
            dma_x(xBs, xb_d, 0)

            # warm_s memset first: the PE warmup depends on it, the big
            # zero-fills can follow
            warm_s = const.tile([P, 512], BF16)
            nc.gpsimd.memset(warm_s[:], 0.25)
            nc.gpsimd.memset(vNat[:, :, 64], 1.0)
            nc.gpsimd.memset(vNat[:, :, 130], 1.0)
            nc.gpsimd.memset(qT[HD:P, :], 0.0)
            nc.gpsimd.memset(k2z[HD:P, :], 0.0)
            nc.gpsimd.memset(cT[HD:P, :], 0.0)

            # ---- PE warmup: ramp the clock while DMAs stream in ----
            for i in range(36):
                wps = ps.tile([P, 512], F32, name="ps", tag="ps")
                w = P if i < 16 else 512
                nc.tensor.matmul(
                    wps[:, 0:w],
                    warm_s[:, 0:P],
                    warm_s[:, 0:w],
                    start=True,
                    stop=True,
                )

            # Second wave: later tile groups, roughly in first-use order.
            dma_x(xTs, xt_d, 1)
            for t in range(2, NT):
                dma_x(xTs, xt_d, t)
                if t % 2 == 0 and t // 2 < 4:
                    dma_x(xBs, xb_d, t // 2)

            # ---- projection pieces for tile group t ----
            def mk_projK(t):
                def f():
                    pp = ps.tile([P, 512], F32, name="ps", tag="ps")
                    for c in range(KC):
                        nc.tensor.matmul(
                            pp[:],
                            wk_s[:, P * c : P * (c + 1)],
                            xTs[:, t, c, :],
                            start=(c == 0),
                            stop=(c == KC - 1),
                        )
                    nc.vector.tensor_copy(k2[:, 512 * t : 512 * (t + 1)], pp[:])
                    nc.vector.tensor_copy(
                        k2z[0:HD, 512 * t : 512 * (t + 1)], pp[0:HD, :]
                    )

                return f

            def mk_projV(t):
                def f():
                    pp = ps.tile([P, 512], F32, name="ps", tag="ps")
                    for c in range(KC):
                        nc.tensor.matmul(
                            pp[:],
                            wv_s[:, P * c : P * (c + 1)],
                            xTs[:, t, c, :],
                            start=(c == 0),
                            stop=(c == KC - 1),
                        )
                    vt_t = vtp.tile([P, 512], BF16, name="vt")
                    nc.vector.tensor_copy(vt_t[:], pp[:])
                    f.vt = vt_t

                return f

            def mk_projQ(t):
                def f():
                    pp = ps.tile([P, 512], F32, name="ps", tag="ps")
                    for c in range(KC):
                        nc.tensor.matmul(
                            pp[0:HD, :],
                            wq_s[:, P * c : P * c + HD],
                            xTs[:, t, c, :],
                            start=(c == 0),
                            stop=(c == KC - 1),
                        )
                    if t % 2 == 0:
                        g = t // 2
                        for c in range(KC):
                            nc.tensor.matmul(
                                pp[HD:P, :],
                                wq_s[:, P * c + HD : P * (c + 1)],
                                xBs[:, g, c, :],
                                start=(c == 0),
                                stop=(c == KC - 1),
                            )
                    nc.vector.tensor_copy(
                        qT[0:HD, 512 * t : 512 * (t + 1)], pp[0:HD, :]
                    )
                    if t % 2 == 0:
                        g = t // 2
                        nc.vector.tensor_copy(
                            qT[HD:P, 512 * g : 512 * (g + 1)], pp[HD:P, :]
                        )

                return f

            def mk_transV(t, projv):
                def f():
                    tp = tpp.tile([P, 4, P], BF16, name="tp")
                    for b in range(4):
                        nc.tensor.transpose(
                            tp[:, b, :],
                            projv.vt[:, P * b : P * (b + 1)],
                            identb,
                        )
                    nc.vector.tensor_copy(
                        vNat[:, 4 * t : 4 * t + 4, 0:HD], tp[:, :, 0:HD]
                    )
                    nc.vector.tensor_copy(
                        vNat[:, 4 * t : 4 * t + 4, 66:130], tp[:, :, HD:P]
                    )

                return f

            def proj_pieces(t):
                pv = mk_projV(t)
                return [mk_projK(t), pv, mk_projQ(t), mk_transV(t, pv)]

            # ---- out-projection: pair an A block (PE rows 0:64) with a
            # B block (rows 64:128) so the two K=64 matmuls co-execute on
            # disjoint array row-halves.
            def mk_outp2(stA, stB):
                def f():
                    osbA = osbp.tile([P, D], BF16, name="osb") \
                        if stA is not None else None
                    osbB = osbp.tile([P, D], BF16, name="osb") \
                        if stB is not None else None
                    for h in range(2):
                        if stA is not None:
                            poA = ps.tile([P, 512], F32, name="ps", tag="ps")
                            if stB is None:
                                # solo: K=128 with zero-padded weights
                                nc.tensor.matmul(
                                    poA[:, 0:384],
                                    cT[:, P * stA : P * (stA + 1)],
                                    woz_s[:, 384 * h : 384 * (h + 1)],
                                    start=True,
                                    stop=True,
                                )
                            else:
                                nc.tensor.matmul(
                                    poA[:, 0:384],
                                    cT[0:HD, P * stA : P * (stA + 1)],
                                    wo_s[0:HD, 384 * h : 384 * (h + 1)],
                                    start=True,
                                    stop=True,
                                )
                        if stB is not None:
                            poB = ps.tile([P, 512], F32, name="ps", tag="ps")
                            nc.tensor.matmul(
                                poB[:, 0:384],
                                cT[HD:P, P * stB : P * (stB + 1)],
                                wo_s[HD:P, 384 * h : 384 * (h + 1)],
                                start=True,
                                stop=True,
                            )
                        if stA is not None:
                            nc.vector.tensor_copy(
                                osbA[:, 384 * h : 384 * (h + 1)], poA[:, 0:384]
                            )
                        if stB is not None:
                            nc.vector.tensor_copy(
                                osbB[:, 384 * h : 384 * (h + 1)], poB[:, 0:384]
                            )
                    if stA is not None:
                        nc.sync.dma_start(
                            outa_d[P * stA : P * (stA + 1), :], osbA[:]
                        )
                    if stB is not None:
                        nc.sync.dma_start(
                            outb_d[P * stB : P * (stB + 1), :], osbB[:]
                        )

                return f

            a_stash = deque()

            def outpB_pieces(T):
                out = []
                for i in range(4):
                    stA = a_stash.popleft() if a_stash else None
                    out.append(mk_outp2(stA, 4 * T + i))
                return out

            def solo_pieces(keep):
                out = []
                while len(a_stash) > keep:
                    out.append(mk_outp2(a_stash.popleft(), None))
                return out

            # ---- attention for tile t; pops bg pieces into PE slack ----
            # Slot B runs 512-wide query tiles (B-tile T spans A-tiles 2T and
            # 2T+1): key blocks 0..8T+3 during the even tile, the 4 diagonal
            # blocks during the odd tile.
            def attn(t, bgP, bgO, bstate):
                T = t // 2
                nkb = 4 * (t + 1)
                ctxA_t = ctxAp.tile([P, 512], F32, name="ctxA")
                if t % 2 == 0:
                    bstate["ctxB"] = ctxBp.tile([P, 512], F32, name="ctxB")
                    b_list = list(range(0, 8 * T + 4))
                else:
                    b_list = list(range(8 * T + 4, 8 * T + 8))
                ctxB_t = bstate["ctxB"]

                def issue_ctxA(pA, r0A, kb):
                    nc.tensor.matmul(
                        ctxA_t[0:65, r0A:512],
                        vNat[:, kb, 0:65],
                        pA[:, r0A:512],
                        start=(kb == 0),
                        stop=(kb == nkb - 1),
                    )

                def issue_ctxB(pB, r0B, kbb):
                    last = kbb == 8 * T + 7
                    nc.tensor.matmul(
                        ctxB_t[0:65, r0B:512],
                        vNat[:, kbb, 66:131],
                        pB[:, r0B:512],
                        start=(kbb == 0),
                        stop=last,
                    )
                    if last:
                        bstate["b_done"] = True

                def flush(entry):
                    pA, r0A, kb, pB, r0B, kbb = entry
                    issue_ctxA(pA, r0A, kb)
                    if pB is not None:
                        issue_ctxB(pB, r0B, kbb)

                bi = 0
                pend = []
                for j in range(0, nkb, 2):
                    if j == 2 and bstate.get("prevA") is not None:
                        # previous tile's normalization, deferred past this
                        # tile's first masks/exps so the DVE chain doesn't
                        # delay them
                        normalize_A(*bstate.pop("prevA"))
                    # --- scores for blocks j, j+1 back-to-back: the PE
                    # 64-deep window pulls LDWEIGHTS ahead, no sem stalls.
                    cur = []
                    for kb in (j, j + 1):
                        d = kb - 4 * t
                        r0A = P * d if d >= 0 else 0
                        scA = ps.tile([P, 512], F32, name="ps", tag="ps")
                        if bi < len(b_list):
                            # paired with a B block: K=64 halves co-execute
                            nc.tensor.matmul(
                                scA[:, r0A:512],
                                k2[0:HD, P * kb : P * (kb + 1)],
                                qT[0:HD, 512 * t + r0A : 512 * (t + 1)],
                                start=True,
                                stop=True,
                            )
                        else:
                            # solo: zero-padded K=128 streams at full rate
                            nc.tensor.matmul(
                                scA[:, r0A:512],
                                k2z[:, P * kb : P * (kb + 1)],
                                qT[:, 512 * t + r0A : 512 * (t + 1)],
                                start=True,
                                stop=True,
                            )
                        if bi < len(b_list):
                            kbb = b_list[bi]
                            bi += 1
                            dB = kbb - 8 * T
                            r0B = HD * dB if dB >= 0 else 0
                            scB = ps.tile([P, 512], F32, name="ps", tag="ps")
                            nc.tensor.matmul(
                                scB[:, r0B:512],
                                k2[HD:P, P * kbb : P * (kbb + 1)],
                                qT[HD:P, 512 * T + r0B : 512 * (T + 1)],
                                start=True,
                                stop=True,
                            )
                        else:
                            scB, r0B, kbb, dB = None, 0, -1, -1
                        cur.append((kb, d, r0A, scA, kbb, dB, r0B, scB))
                    # --- masks + exps for the pair
                    for kb, d, r0A, scA, kbb, dB, r0B, scB in cur:
                        if d >= 0:
                            nc.vector.tensor_tensor(
                                scA[:, r0A : r0A + P],
                                scA[:, r0A : r0A + P],
                                ma_s,
                                ADD,
                            )
                        pA = ptp.tile([P, 512], BF16, name="pt", tag="pt")
                        nc.scalar.activation(
                            pA[:, r0A:512], scA[:, r0A:512], AF.Exp, scale=0.125
                        )
                        pB = None
                        if scB is not None:
                            if dB >= 0:
                                nc.vector.tensor_tensor(
                                    scB[:, r0B : r0B + HD],
                                    scB[:, r0B : r0B + HD],
                                    mb_s,
                                    ADD,
                                )
                            pB = ptp.tile([P, 512], BF16, name="pt", tag="pt")
                            nc.scalar.activation(
                                pB[:, r0B:512],
                                scB[:, r0B:512],
                                AF.Exp,
                                scale=0.125,
                            )
                        pend.append((pA, r0A, kb, pB, r0B, kbb))
                    # --- ctx for the PREVIOUS pair (exps long done)
                    while len(pend) > 2:
                        flush(pend.pop(0))
                    if bstate.get("b_done"):
                        bstate["b_done"] = False
                        normalize_B(T, bstate)
                        bgO.extend(outpB_pieces(T))
                    # --- background pieces spread over remaining pairs
                    rem = (nkb - j) // 2
                    if bgP:
                        npop = min(len(bgP), max(1, -(-len(bgP) // rem)))
                        for _ in range(npop):
                            bgP.popleft()()
                    elif bgO and j % 2 == 0:
                        bgO.popleft()()
                while bgP:
                    bgP.popleft()()
                while pend:
                    flush(pend.pop(0))
                if bstate.get("b_done"):
                    bstate["b_done"] = False
                    normalize_B(T, bstate)
                    bgO.extend(outpB_pieces(T))
                return ctxA_t

            def normalize_A(t, ctxA_t):
                # reciprocal_approx_fast mis-reads PSUM at partition offsets;
                # stage l into SBUF partition 0 first (plain DVE ops rebase
                # partitions correctly).
                lsA = sm.tile([1, 512], F32, name="lsA")
                nc.vector.tensor_copy(lsA[:], ctxA_t[64:65, :])
                lrA = sm.tile([1, 512], F32, name="lrA")
                nc.vector.reciprocal_approx_fast(lrA[:], lsA[:])
                lbA = sm.tile([HD, 512], F32, name="lbA")
                nc.gpsimd.partition_broadcast(lbA[:], lrA[0:1, :])
                nc.vector.tensor_tensor(
                    cT[0:HD, 512 * t : 512 * (t + 1)],
                    ctxA_t[0:HD, :],
                    lbA[:],
                    MULT,
                )

            def normalize_B(T, bstate):
                ctxB_t = bstate["ctxB"]
                lsB = sm.tile([1, 512], F32, name="lsB")
                nc.vector.tensor_copy(lsB[:], ctxB_t[64:65, :])
                lrB = sm.tile([1, 512], F32, name="lrB")
                nc.vector.reciprocal_approx_fast(lrB[:], lsB[:])
                lbB = sm.tile([HD, 512], F32, name="lbB")
                nc.gpsimd.partition_broadcast(lbB[:], lrB[0:1, :])
                nc.vector.tensor_tensor(
                    cT[HD:P, 512 * T : 512 * (T + 1)],
                    ctxB_t[0:HD, :],
                    lbB[:],
                    MULT,
                )

            # ---- main schedule ----
            bgP = deque()  # projection pieces: must complete within the tile
            bgO = deque()  # out-projection pieces: carry across tiles
            bstate = {}
            for p in proj_pieces(0):
                p()
            for t in range(NT):
                if t < NT - 1:
                    bgP.extend(proj_pieces(t + 1))
                ctxA_t = attn(t, bgP, bgO, bstate)
                bstate["prevA"] = (t, ctxA_t)
                a_stash.extend(4 * t + i for i in range(4))
                bgO.extend(solo_pieces(4))
            normalize_A(*bstate.pop("prevA"))
            bgO.extend(solo_pieces(0))
            while bgO:
                bgO.popleft()()

    nc.compile()
    return nc


def _host_inputs(x, W_query, W_key, W_value, W_out):
    bf = ml_dtypes.bfloat16
    x2 = np.asarray(x, np.float32).reshape(S, D)
    xT = np.ascontiguousarray(x2.T).astype(bf)  # [768, 4096]
    xt8 = np.ascontiguousarray(
        xT.reshape(KC, P, NT, 512).transpose(2, 1, 0, 3)
    ).reshape(8 * P, KC, 512)
    xb8 = []
    for par in range(2):
        xbT = np.ascontiguousarray(x2[par::2].T).astype(bf)  # [768, 2048]
        xb8.append(
            np.ascontiguousarray(
                xbT.reshape(KC, P, 4, 512).transpose(2, 1, 0, 3)
            ).reshape(4 * P, KC, 512)
        )
    ii, jj = np.arange(P)[:, None], np.arange(P)[None, :]
    ma = np.where(ii > jj, NEG, 0.0).astype(np.float32)
    jb = np.arange(HD)[None, :]
    mb = [
        np.where(ii > 2 * jb + par, NEG, 0.0).astype(np.float32)
        for par in range(2)
    ]
    ident = np.eye(P, dtype=bf)

    def wslice(w, h):
        return np.asarray(w, np.float32)[:, HD * h : HD * (h + 1)]

    in_maps = []
    for core in range(8):
        ha, hb, par = core, 8 + core // 2, core % 2
        wq = np.concatenate([wslice(W_query, ha), wslice(W_query, hb)], axis=1)
        wk = np.concatenate([wslice(W_key, ha), wslice(W_key, hb)], axis=1)
        wv = np.concatenate([wslice(W_value, ha), wslice(W_value, hb)], axis=1)
        wo = np.concatenate(
            [
                np.asarray(W_out, np.float32)[HD * ha : HD * (ha + 1), :],
                np.asarray(W_out, np.float32)[HD * hb : HD * (hb + 1), :],
            ],
            axis=0,
        )
        def wpack(w):
            return w.astype(bf).reshape(KC, P, P).transpose(1, 0, 2).reshape(P, D)

        woz = np.concatenate(
            [wo[0:HD, :], np.zeros((HD, D), np.float32)], axis=0
        )
        cst = np.concatenate(
            [
                ident,
                ma.astype(bf),
                mb[par].astype(bf),
                wpack(wq),
                wpack(wk),
                wpack(wv),
                wo.astype(bf),
                woz.astype(bf),
            ],
            axis=1,
        )
        in_maps.append(
            {
                "xt": xt8,
                "xb": xb8[par],
                "cst": np.ascontiguousarray(cst),
            }
        )
    return in_maps


def run(x, W_query, W_key, W_value, W_out, b_out, trace=False):
    global _CACHED_NC
    if _CACHED_NC is None:
        _CACHED_NC = build_nc()
    nc = _CACHED_NC
    in_maps = _host_inputs(x, W_query, W_key, W_value, W_out)
    res = run_bass_kernel_spmd(nc, in_maps, core_ids=list(range(8)), trace=trace)
    out = np.zeros((S, D), dtype=np.float32)
    for core in range(8):
        out += np.asarray(res.results[core]["outA"], dtype=np.float32)
    for core in range(8):
        par = core % 2
        out[par::2] += np.asarray(res.results[core]["outB"], dtype=np.float32)
    out += np.asarray(b_out, np.float32)[None, :]
    return out, res


def kernel(x, W_query, W_key, W_value, W_out, b_out):
    out, _ = run(
        np.asarray(x, np.float32).reshape(S, D),
        np.asarray(W_query, np.float32),
        np.asarray(W_key, np.float32),
        np.asarray(W_value, np.float32),
        np.asarray(W_out, np.float32),
        np.asarray(b_out, np.float32),
    )
    return out.reshape(1, S, D)


# revision 33
# speedup vs baseline: 1.0085x; 1.0085x over previous
"""Causal multi-head attention (B=1, S=4096, D=768, H=12, d_head=64) on 8
Trainium2 NeuronCores.

Sharding: exact 1.5 heads per core. Slot A = head c (c = core id 0..7), full
causal attention over all 4096 queries. Slot B = head 8 + c//2 restricted to
query tokens of parity c%2 (2048 alternate tokens, full key range), so the 4
remaining heads are each split across two cores by query parity with zero
duplicated work and a uniform SPMD program (the parity lives in the data:
host-gathered xB rows and a parity-dependent boundary mask).

All matmul operands are bf16 (PSUM accumulation stays f32); the host supplies
x already transposed (and parity-gathered for slot B), so the device does no
x transposes at all. Per query tile the kernel interleaves next-tile QKV
projections and previous-tile out-projections into the attention block loop to
keep the PE busy (and at full clock) while the Scalar engine runs the exps.
Softmax denominators come free via ones-columns appended to V; normalization
uses reciprocal_approx_fast on DVE; out-proj PSUM->SBUF copies run on DVE.
Partial outputs are written bf16; the host sums them (the all-reduce of the
row-parallel out projection) and adds b_out.
"""

import sys

sys.path.insert(0, "/opt/trn_rl_repo")

from collections import deque

import ml_dtypes
import numpy as np

import concourse.bass as bass
import concourse.tile as tile
from concourse import bacc, mybir
from concourse.bass_utils import run_bass_kernel_spmd

S = 4096
D = 768
HD = 64
P = 128
KC = D // P  # 6 contraction chunks for the projections
NT = 8  # 512-token query tiles
NEG = -1e30

F32 = mybir.dt.float32
BF16 = mybir.dt.bfloat16
AF = mybir.ActivationFunctionType
ADD = mybir.AluOpType.add
MULT = mybir.AluOpType.mult

_CACHED_NC = None


def build_nc():
    nc = bacc.Bacc("TRN2", target_bir_lowering=False, debug=False, num_devices=8)

    xt_d = nc.declare_dram_parameter("xt", [8 * P, KC, 512], BF16, isOutput=False)
    xb_d = nc.declare_dram_parameter("xb", [4 * P, KC, 512], BF16, isOutput=False)
    # all constants packed in one bf16 tensor: [ident 128 | ma 128 | mb 64 |
    # wq 768 | wk 768 | wv 768 | wo 768 | woz 768] = 4160 columns -> one DMA
    # wave of 128 descriptors (descriptor dispatch is ~14.5ns each); woz is
    # [woA ; zeros] for full-rate K=128 solo out-projections
    cst_d = nc.declare_dram_parameter("cst", [P, 4160], BF16, isOutput=False)
    outa_d = nc.declare_dram_parameter("outA", [S, D], BF16, isOutput=True)
    outb_d = nc.declare_dram_parameter("outB", [S // 2, D], BF16, isOutput=True)

    with tile.TileContext(nc) as tc:
        with (
            tc.tile_pool(name="const", bufs=1) as const,
            tc.tile_pool(name="big", bufs=1) as big,
            tc.tile_pool(name="pt", bufs=8) as ptp,
            tc.tile_pool(name="vt", bufs=2) as vtp,
            tc.tile_pool(name="osb", bufs=4) as osbp,
            tc.tile_pool(name="sm", bufs=2) as sm,
            tc.tile_pool(name="ps", bufs=4, space="PSUM") as ps,
            tc.tile_pool(name="ctxA", bufs=2, space="PSUM") as ctxAp,
            tc.tile_pool(name="ctxB", bufs=1, space="PSUM") as ctxBp,
            tc.tile_pool(name="tpp", bufs=1, space="PSUM") as tpp,
        ):
            # ---- constants ----
            # Split each load 4 ways by partition: one queue processes
            # descriptors serially (~150ns each), so a 128-partition DMA on a
            # single queue takes ~19us; 4 queues cut that to ~5us.
            def dma4(dst, src):
                for q4 in range(4):
                    nc.sync.dma_start(dst[32 * q4 : 32 * (q4 + 1)],
                                      src[32 * q4 : 32 * (q4 + 1)])

            cst_s = const.tile([P, 4160], BF16)
            identb = cst_s[:, 0:128]
            ma_s = cst_s[:, 128:256]
            mb_s = cst_s[:, 256:320]
            wq_s = cst_s[:, 320:1088]
            wk_s = cst_s[:, 1088:1856]
            wv_s = cst_s[:, 1856:2624]
            wo_s = cst_s[:, 2624:3392]
            woz_s = cst_s[:, 3392:4160]

            # ---- persistent activations ----
            xTs = big.tile([P, NT, KC, 512], BF16)  # x^T, group-major
            xBs = big.tile([P, 4, KC, 512], BF16)  # x^T of slot-B tokens
            qT = big.tile([P, S], BF16)  # rows 0:64 qA^T, 64:128 qB^T (cols 0:2048)
            k2 = big.tile([P, S], BF16)  # rows 0:64 kA^T, 64:128 kB^T
            # v natural per 128-key block: cols 0:64 vA, 64 ones, 66:130 vB,
            # 130 ones (65/131 unused)
            vNat = big.tile([P, S // P, 132], BF16)
            cT = big.tile([P, S], BF16)  # rows 0:64 ctxA^T, 64:128 ctxB^T
            # kA with zeroed upper rows: solo A-scores contract K=128 at full
            # stream rate (K=64 matmuls stream at half rate unless paired)
            k2z = big.tile([P, S], BF16)

            def dma_x(dst, src, g):
                for q4 in range(4):
                    nc.sync.dma_start(
                        dst[32 * q4 : 32 * (q4 + 1), g, :, :],
                        src[P * g + 32 * q4 : P * g + 32 * (q4 + 1), :, :],
                    )

            # Critical wave: everything tile group 0 needs, first in the
            # queues.
            dma4(cst_s, cst_d)
            dma_x(xTs, xt_d, 0)
            dma_x(xBs, xb_d, 0)

            # warm_s memset first: the PE warmup depends on it, the big
            # zero-fills can follow
            warm_s = const.tile([P, 512], BF16)
            nc.gpsimd.memset(warm_s[:], 0.25)
            nc.gpsimd.memset(vNat[:, :, 64], 1.0)
            nc.gpsimd.memset(vNat[:, :, 130], 1.0)
            nc.gpsimd.memset(qT[HD:P, :], 0.0)
            nc.gpsimd.memset(k2z[HD:P, :], 0.0)
            nc.gpsimd.memset(cT[HD:P, :], 0.0)

            # ---- PE warmup: ramp the clock while DMAs stream in ----
            for i in range(36):
                wps = ps.tile([P, 512], F32, name="ps", tag="ps")
                w = P if i < 16 else 512
                nc.tensor.matmul(
                    wps[:, 0:w],
                    warm_s[:, 0:P],
                    warm_s[:, 0:w],
                    start=True,
                    stop=True,
                )

            # Second wave: later tile groups, roughly in first-use order.
            dma_x(xTs, xt_d, 1)
            for t in range(2, NT):
                dma_x(xTs, xt_d, t)
                if t % 2 == 0 and t // 2 < 4:
                    dma_x(xBs, xb_d, t // 2)

            # ---- projection pieces for tile group t ----
            def mk_projK(t):
                def f():
                    pp = ps.tile([P, 512], F32, name="ps", tag="ps")
                    for c in range(KC):
                        nc.tensor.matmul(
                            pp[:],
                            wk_s[:, P * c : P * (c + 1)],
                            xTs[:, t, c, :],
                            start=(c == 0),
                            stop=(c == KC - 1),
                        )
                    nc.vector.tensor_copy(k2[:, 512 * t : 512 * (t + 1)], pp[:])
                    nc.vector.tensor_copy(
                        k2z[0:HD, 512 * t : 512 * (t + 1)], pp[0:HD, :]
                    )

                return f

            def mk_projV(t):
                def f():
                    pp = ps.tile([P, 512], F32, name="ps", tag="ps")
                    for c in range(KC):
                        nc.tensor.matmul(
                            pp[:],
                            wv_s[:, P * c : P * (c + 1)],
                            xTs[:, t, c, :],
                            start=(c == 0),
                            stop=(c == KC - 1),
                        )
                    vt_t = vtp.tile([P, 512], BF16, name="vt")
                    nc.vector.tensor_copy(vt_t[:], pp[:])
                    f.vt = vt_t

                return f

            def mk_projQ(t):
                def f():
                    pp = ps.tile([P, 512], F32, name="ps", tag="ps")
                    for c in range(KC):
                        nc.tensor.matmul(
                            pp[0:HD, :],
                            wq_s[:, P * c : P * c + HD],
                            xTs[:, t, c, :],
                            start=(c == 0),
                            stop=(c == KC - 1),
                        )
                    if t % 2 == 0:
                        g = t // 2
                        for c in range(KC):
                            nc.tensor.matmul(
                                pp[HD:P, :],
                                wq_s[:, P * c + HD : P * (c + 1)],
                                xBs[:, g, c, :],
                                start=(c == 0),
                                stop=(c == KC - 1),
                            )
                    nc.vector.tensor_copy(
                        qT[0:HD, 512 * t : 512 * (t + 1)], pp[0:HD, :]
                    )
                    if t % 2 == 0:
                        g = t // 2
                        nc.vector.tensor_copy(
                            qT[HD:P, 512 * g : 512 * (g + 1)], pp[HD:P, :]
                        )

                return f

            def mk_transV(t, projv):
                def f():
                    tp = tpp.tile([P, 4, P], BF16, name="tp")
                    for b in range(4):
                        nc.tensor.transpose(
                            tp[:, b, :],
                            projv.vt[:, P * b : P * (b + 1)],
                            identb,
                        )
                    nc.vector.tensor_copy(
                        vNat[:, 4 * t : 4 * t + 4, 0:HD], tp[:, :, 0:HD]
                    )
                    nc.vector.tensor_copy(
                        vNat[:, 4 * t : 4 * t + 4, 66:130], tp[:, :, HD:P]
                    )

                return f

            def proj_pieces(t):
                pv = mk_projV(t)
                return [mk_projK(t), pv, mk_projQ(t), mk_transV(t, pv)]

            # ---- out-projection: pair an A block (PE rows 0:64) with a
            # B block (rows 64:128) so the two K=64 matmuls co-execute on
            # disjoint array row-halves.
            def mk_outp2(stA, stB):
                def f():
                    osbA = osbp.tile([P, D], BF16, name="osb") \
                        if stA is not None else None
                    osbB = osbp.tile([P, D], BF16, name="osb") \
                        if stB is not None else None
                    for h in range(2):
                        if stA is not None:
                            poA = ps.tile([P, 512], F32, name="ps", tag="ps")
                            if stB is None:
                                # solo: K=128 with zero-padded weights
                                nc.tensor.matmul(
                                    poA[:, 0:384],
                                    cT[:, P * stA : P * (stA + 1)],
                                    woz_s[:, 384 * h : 384 * (h + 1)],
                                    start=True,
                                    stop=True,
                                )
                            else:
                                nc.tensor.matmul(
                                    poA[:, 0:384],
                                    cT[0:HD, P * stA : P * (stA + 1)],
                                    wo_s[0:HD, 384 * h : 384 * (h + 1)],
                                    start=True,
                                    stop=True,
                                )
                        if stB is not None:
                            poB = ps.tile([P, 512], F32, name="ps", tag="ps")
                            nc.tensor.matmul(
                                poB[:, 0:384],
                                cT[HD:P, P * stB : P * (stB + 1)],
                                wo_s[HD:P, 384 * h : 384 * (h + 1)],
                                start=True,
                                stop=True,
                            )
                        if stA is not None:
                            nc.vector.tensor_copy(
                                osbA[:, 384 * h : 384 * (h + 1)], poA[:, 0:384]
                            )
                        if stB is not None:
                            nc.vector.tensor_copy(
                                osbB[:, 384 * h : 384 * (h + 1)], poB[:, 0:384]
                            )
                    if stA is not None:
                        nc.sync.dma_start(
                            outa_d[P * stA : P * (stA + 1), :], osbA[:]
                        )
                    if stB is not None:
                        nc.sync.dma_start(
                            outb_d[P * stB : P * (stB + 1), :], osbB[:]
                        )

                return f

            a_stash = deque()

            def outpB_pieces(T):
                out = []
                for i in range(4):
                    stA = a_stash.popleft() if a_stash else None
                    out.append(mk_outp2(stA, 4 * T + i))
                return out

            def solo_pieces(keep):
                out = []
                while len(a_stash) > keep:
                    out.append(mk_outp2(a_stash.popleft(), None))
                return out

            # ---- attention for tile t; pops bg pieces into PE slack ----
            # Slot B runs 512-wide query tiles (B-tile T spans A-tiles 2T and
            # 2T+1): key blocks 0..8T+3 during the even tile, the 4 diagonal
            # blocks during the odd tile.
            def attn(t, bgP, bgO, bstate):
                T = t // 2
                nkb = 4 * (t + 1)
                ctxA_t = ctxAp.tile([P, 512], F32, name="ctxA")
                if t % 2 == 0:
                    bstate["ctxB"] = ctxBp.tile([P, 512], F32, name="ctxB")
                    b_list = list(range(0, 8 * T + 4))
                else:
                    b_list = list(range(8 * T + 4, 8 * T + 8))
                ctxB_t = bstate["ctxB"]

                def issue_ctxA(pA, r0A, kb):
                    nc.tensor.matmul(
                        ctxA_t[0:65, r0A:512],
                        vNat[:, kb, 0:65],
                        pA[:, r0A:512],
                        start=(kb == 0),
                        stop=(kb == nkb - 1),
                    )

                def issue_ctxB(pB, r0B, kbb):
                    last = kbb == 8 * T + 7
                    nc.tensor.matmul(
                        ctxB_t[0:65, r0B:512],
                        vNat[:, kbb, 66:131],
                        pB[:, r0B:512],
                        start=(kbb == 0),
                        stop=last,
                    )
                    if last:
                        bstate["b_done"] = True

                def flush(entry):
                    pA, r0A, kb, pB, r0B, kbb = entry
                    issue_ctxA(pA, r0A, kb)
                    if pB is not None:
                        issue_ctxB(pB, r0B, kbb)

                bi = 0
                pend = []
                for j in range(0, nkb, 2):
                    if j == 2 and bstate.get("prevA") is not None:
                        # previous tile's normalization, deferred past this
                        # tile's first masks/exps so the DVE chain doesn't
                        # delay them
                        normalize_A(*bstate.pop("prevA"))
                    # --- scores for blocks j, j+1 back-to-back: the PE
                    # 64-deep window pulls LDWEIGHTS ahead, no sem stalls.
                    cur = []
                    for kb in (j, j + 1):
                        d = kb - 4 * t
                        r0A = P * d if d >= 0 else 0
                        scA = ps.tile([P, 512], F32, name="ps", tag="ps")
                        if bi < len(b_list):
                            # paired with a B block: K=64 halves co-execute
                            nc.tensor.matmul(
                                scA[:, r0A:512],
                                k2[0:HD, P * kb : P * (kb + 1)],
                                qT[0:HD, 512 * t + r0A : 512 * (t + 1)],
                                start=True,
                                stop=True,
                            )
                        else:
                            # solo: zero-padded K=128 streams at full rate
                            nc.tensor.matmul(
                                scA[:, r0A:512],
                                k2z[:, P * kb : P * (kb + 1)],
                                qT[:, 512 * t + r0A : 512 * (t + 1)],
                                start=True,
                                stop=True,
                            )
                        if bi < len(b_list):
                            kbb = b_list[bi]
                            bi += 1
                            dB = kbb - 8 * T
                            r0B = HD * dB if dB >= 0 else 0
                            scB = ps.tile([P, 512], F32, name="ps", tag="ps")
                            nc.tensor.matmul(
                                scB[:, r0B:512],
                                k2[HD:P, P * kbb : P * (kbb + 1)],
                                qT[HD:P, 512 * T + r0B : 512 * (T + 1)],
                                start=True,
                                stop=True,
                            )
                        else:
                            scB, r0B, kbb, dB = None, 0, -1, -1
                        cur.append((kb, d, r0A, scA, kbb, dB, r0B, scB))
                    # --- masks + exps for the pair
                    for kb, d, r0A, scA, kbb, dB, r0B, scB in cur:
                        if d >= 0:
                            nc.vector.tensor_tensor(
                                scA[:, r0A : r0A + P],
                                scA[:, r0A : r0A + P],
                                ma_s,
                                ADD,
                            )
                        pA = ptp.tile([P, 512], BF16, name="pt", tag="pt")
                        nc.scalar.activation(
                            pA[:, r0A:512], scA[:, r0A:512], AF.Exp, scale=0.125
                        )
                        pB = None
                        if scB is not None:
                            if dB >= 0:
                                nc.vector.tensor_tensor(
                                    scB[:, r0B : r0B + HD],
                                    scB[:, r0B : r0B + HD],
                                    mb_s,
                                    ADD,
                                )
                            pB = ptp.tile([P, 512], BF16, name="pt", tag="pt")
                            nc.scalar.activation(
                                pB[:, r0B:512],
                                scB[:, r0B:512],
                                AF.Exp,
                                scale=0.125,
                            )
                        pend.append((pA, r0A, kb, pB, r0B, kbb))
                    # --- ctx for the PREVIOUS pair (exps long done)
                    while len(pend) > 2:
                        flush(pend.pop(0))
                    if bstate.get("b_done"):
                        bstate["b_done"] = False
                        normalize_B(T, bstate)
                        bgO.extend(outpB_pieces(T))
                    # --- background pieces spread over remaining pairs
                    rem = (nkb - j) // 2
                    if bgP:
                        npop = min(len(bgP), max(1, -(-len(bgP) // rem)))
                        for _ in range(npop):
                            bgP.popleft()()
                    elif bgO and j % 4 == 0:
                        bgO.popleft()()
                while bgP:
                    bgP.popleft()()
                while pend:
                    flush(pend.pop(0))
                if bstate.get("b_done"):
                    bstate["b_done"] = False
                    normalize_B(T, bstate)
                    bgO.extend(outpB_pieces(T))
                return ctxA_t

            def normalize_A(t, ctxA_t):
                # reciprocal_approx_fast mis-reads PSUM at partition offsets;
                # stage l into SBUF partition 0 first (plain DVE ops rebase
                # partitions correctly).
                lsA = sm.tile([1, 512], F32, name="lsA")
                nc.vector.tensor_copy(lsA[:], ctxA_t[64:65, :])
                lrA = sm.tile([1, 512], F32, name="lrA")
                nc.vector.reciprocal_approx_fast(lrA[:], lsA[:])
                lbA = sm.tile([HD, 512], F32, name="lbA")
                nc.gpsimd.partition_broadcast(lbA[:], lrA[0:1, :])
                nc.vector.tensor_tensor(
                    cT[0:HD, 512 * t : 512 * (t + 1)],
                    ctxA_t[0:HD, :],
                    lbA[:],
                    MULT,
                )

            def normalize_B(T, bstate):
                ctxB_t = bstate["ctxB"]
                lsB = sm.tile([1, 512], F32, name="lsB")
                nc.vector.tensor_copy(lsB[:], ctxB_t[64:65, :])
                lrB = sm.tile([1, 512], F32, name="lrB")
                nc.vector.reciprocal_approx_fast(lrB[:], lsB[:])
                lbB = sm.tile([HD, 512], F32, name="lbB")
                nc.gpsimd.partition_broadcast(lbB[:], lrB[0:1, :])
                nc.vector.tensor_tensor(
                    cT[HD:P, 512 * T : 512 * (T + 1)],
                    ctxB_t[0:HD, :],
                    lbB[:],
                    MULT,
                )

            # ---- main schedule ----
            bgP = deque()  # projection pieces: must complete within the tile
            bgO = deque()  # out-projection pieces: carry across tiles
            bstate = {}
            for p in proj_pieces(0):
                p()
            for t in range(NT):
                if t < NT - 1:
                    bgP.extend(proj_pieces(t + 1))
                ctxA_t = attn(t, bgP, bgO, bstate)
                bstate["prevA"] = (t, ctxA_t)
                a_stash.extend(4 * t + i for i in range(4))
                bgO.extend(solo_pieces(4))
            normalize_A(*bstate.pop("prevA"))
            bgO.extend(solo_pieces(0))
            while bgO:
                bgO.popleft()()

    nc.compile()
    return nc


def _host_inputs(x, W_query, W_key, W_value, W_out):
    bf = ml_dtypes.bfloat16
    x2 = np.asarray(x, np.float32).reshape(S, D)
    xT = np.ascontiguousarray(x2.T).astype(bf)  # [768, 4096]
    xt8 = np.ascontiguousarray(
        xT.reshape(KC, P, NT, 512).transpose(2, 1, 0, 3)
    ).reshape(8 * P, KC, 512)
    xb8 = []
    for par in range(2):
        xbT = np.ascontiguousarray(x2[par::2].T).astype(bf)  # [768, 2048]
        xb8.append(
            np.ascontiguousarray(
                xbT.reshape(KC, P, 4, 512).transpose(2, 1, 0, 3)
            ).reshape(4 * P, KC, 512)
        )
    ii, jj = np.arange(P)[:, None], np.arange(P)[None, :]
    ma = np.where(ii > jj, NEG, 0.0).astype(np.float32)
    jb = np.arange(HD)[None, :]
    mb = [
        np.where(ii > 2 * jb + par, NEG, 0.0).astype(np.float32)
        for par in range(2)
    ]
    ident = np.eye(P, dtype=bf)

    def wslice(w, h):
        return np.asarray(w, np.float32)[:, HD * h : HD * (h + 1)]

    in_maps = []
    for core in range(8):
        ha, hb, par = core, 8 + core // 2, core % 2
        wq = np.concatenate([wslice(W_query, ha), wslice(W_query, hb)], axis=1)
        wk = np.concatenate([wslice(W_key, ha), wslice(W_key, hb)], axis=1)
        wv = np.concatenate([wslice(W_value, ha), wslice(W_value, hb)], axis=1)
        wo = np.concatenate(
            [
                np.asarray(W_out, np.float32)[HD * ha : HD * (ha + 1), :],
                np.asarray(W_out, np.float32)[HD * hb : HD * (hb + 1), :],
            ],
            axis=0,
        )
        def wpack(w):
            return w.astype(bf).reshape(KC, P, P).transpose(1, 0, 2).reshape(P, D)

        woz = np.concatenate(
            [wo[0:HD, :], np.zeros((HD, D), np.float32)], axis=0
        )
        cst = np.concatenate(
            [
                ident,
                ma.astype(bf),
                mb[par].astype(bf),
                wpack(wq),
                wpack(wk),
                wpack(wv),
                wo.astype(bf),
                woz.astype(bf),
            ],
            axis=1,
        )
        in_maps.append(
            {
                "xt": xt8,
                "xb": xb8[par],
                "cst": np.ascontiguousarray(cst),
            }
        )
    return in_maps


def run(x, W_query, W_key, W_value, W_out, b_out, trace=False):
    global _CACHED_NC
    if _CACHED_NC is None:
        _CACHED_NC = build_nc()
    nc = _CACHED_NC
    in_maps = _host_inputs(x, W_query, W_key, W_value, W_out)
    res = run_bass_kernel_spmd(nc, in_maps, core_ids=list(range(8)), trace=trace)
    out = np.zeros((S, D), dtype=np.float32)
    for core in range(8):
        out += np.asarray(res.results[core]["outA"], dtype=np.float32)
    for core in range(8):
        par = core % 2
        out[par::2] += np.asarray(res.results[core]["outB"], dtype=np.float32)
    out += np.asarray(b_out, np.float32)[None, :]
    return out, res


def kernel(x, W_query, W_key, W_value, W_out, b_out):
    out, _ = run(
        np.asarray(x, np.float32).reshape(S, D),
        np.asarray(W_query, np.float32),
        np.asarray(W_key, np.float32),
        np.asarray(W_value, np.float32),
        np.asarray(W_out, np.float32),
        np.asarray(b_out, np.float32),
    )
    return out.reshape(1, S, D)
